# revision 2
# baseline (speedup 1.0000x reference)
"""Trainium2 Bass kernel for AdaAttentionalPropagation (masked multi-head
cross-attention + merge conv + MLP with InstanceNorm/ReLU).

Full inputs in, full output out. Internally: data-parallel over batch B=8
across 8 NeuronCores (one batch element per core, no collectives).

Math notes (host-side folds, all exact):
  - head channels are re-permuted to blocked layout (h*64+d) by permuting
    Wq/Wk/Wv rows and Wm columns
  - 2^7/(sqrt(dh)*ln2) is folded into Wq and bq, so the masked score is
    already in "128*log2" units
  - bv folds into an effective merge bias bmE = Wm@bv + bm (softmax rows sum
    to 1, so v's bias contributes Wm@bv to the message)
  - b1 is dropped: a per-channel constant cancels in InstanceNorm(affine=False)
  - exp is computed with the Schraudolph bit trick: i16 = t + (127-c)*128,
    bitcast int16 -> bf16 gives ~2^(t/128 - c); the uniform 2^-c factor
    cancels in the softmax normalization.  Two pipeline variants keep the
    elementwise work balanced across ACT/DVE/GPSIMD:
      chain E: ACT evicts scores PSUM->SBUF bf16; DVE multiplies by the mask
               in its 2x bf16 mode; the magic add runs on DVE in 4x mode
               (bf16 in, int16 out)
      chain F: DVE multiplies straight from PSUM (1x); the magic add runs on
               GPSIMD (which cannot touch PSUM but is idle otherwise)
  - softmax denominator comes free from a ones-column appended to v^T in the
    attention matmul (row 64 of the PSUM accumulator)
"""

import sys

for _p in ("/opt/trn_rl_repo", "/root/.axon_site/_ro/trn_rl_repo"):
    if _p not in sys.path:
        sys.path.append(_p)

import numpy as np
import ml_dtypes
from contextlib import ExitStack

import concourse.bass as bass
import concourse.tile as tile
from concourse import bacc, mybir
from concourse.bass_utils import run_bass_kernel_spmd

B, D, N, NKV, H = 8, 256, 2048, 2048, 4
DH = D // H
EPS = 1e-5
NCORES = 8

BF = mybir.dt.bfloat16
F32 = mybir.dt.float32
I16 = mybir.dt.int16
AF = mybir.ActivationFunctionType
ALU = mybir.AluOpType
NPBF = ml_dtypes.bfloat16

# Schraudolph: i16 = t + (127 - C_SHIFT)*128, bitcast to bf16 ~= 2^(t/128-c)
C_SHIFT = 0.0579
MAGIC = (127.0 - C_SHIFT) * 128.0

# per-pass chain split: mc tiles with (mc % 16) in F_SET take chain F
# (mid-pass positions: the slow direct-PSUM multiply must not coincide
# with the pass-boundary handoff the DVE is already absorbing at mc 0)
F_SET = frozenset((4, 12))
# magic-add engine for chain-F tiles: gpsimd turned out to be ~16x slower
# than DVE on tensor_scalar AND to starve the DVE via the shared SBUF port
F_MAGIC_GP = False
# fast-reciprocal custom op for the softmax denominator
USE_FAST_RECIP = False

_CACHE = {}


def _build():
    nc = bacc.Bacc("TRN2", target_bir_lowering=False, debug=False,
                   num_devices=NCORES)

    d_x = nc.dram_tensor("x", [128, 2, N], BF, kind="ExternalInput")
    d_src = nc.dram_tensor("src", [128, 2, N], BF, kind="ExternalInput")
    d_mask = nc.dram_tensor("maskT", [128, 16, N], BF, kind="ExternalInput")
    d_wq = nc.dram_tensor("wqT", [128, 2, 256], BF, kind="ExternalInput")
    d_wk = nc.dram_tensor("wkT", [128, 2, 256], BF, kind="ExternalInput")
    d_wv = nc.dram_tensor("wvT", [128, 2, 256], BF, kind="ExternalInput")
    d_wm = nc.dram_tensor("wmT", [128, 2, 256], BF, kind="ExternalInput")
    d_w1 = nc.dram_tensor("w1T", [128, 4, 512], BF, kind="ExternalInput")
    d_w2 = nc.dram_tensor("w2T", [128, 4, 256], BF, kind="ExternalInput")
    d_bq = nc.dram_tensor("bq", [128, 2], F32, kind="ExternalInput")
    d_bk = nc.dram_tensor("bk", [128, 2], F32, kind="ExternalInput")
    d_bm = nc.dram_tensor("bmE", [128, 2], F32, kind="ExternalInput")
    d_out = nc.dram_tensor("out", [128, 2, N], F32, kind="ExternalOutput")
    d_rscr = nc.dram_tensor("rscratch", [16, 512], F32)
    d_sums = nc.dram_tensor("sscratch", [16, 512], F32)

    with tile.TileContext(nc) as tc, ExitStack() as ctx:
        consts = ctx.enter_context(tc.tile_pool(name="consts", bufs=1))
        spbp = ctx.enter_context(tc.tile_pool(name="spbp", bufs=5))
        ptp = ctx.enter_context(tc.tile_pool(name="ptp", bufs=3))
        pip = ctx.enter_context(tc.tile_pool(name="pip", bufs=3))
        recp = ctx.enter_context(tc.tile_pool(name="recp", bufs=2))
        rbb = ctx.enter_context(tc.tile_pool(name="rbb", bufs=2))
        stgp = ctx.enter_context(tc.tile_pool(name="stgp", bufs=2))
        statp = ctx.enter_context(tc.tile_pool(name="statp", bufs=8))
        outp = ctx.enter_context(tc.tile_pool(name="outp", bufs=2))

        wq_sb = consts.tile([128, 2, 256], BF)
        wk_sb = consts.tile([128, 2, 256], BF)
        wv_sb = consts.tile([128, 2, 256], BF)
        wm_sb = consts.tile([128, 2, 256], BF)
        w1_sb = consts.tile([128, 4, 512], BF)
        w2_sb = consts.tile([128, 4, 256], BF)
        bq_sb = consts.tile([128, 2], F32)
        bk_sb = consts.tile([128, 2], F32)
        bm_sb = consts.tile([128, 2], F32)
        x_sb = consts.tile([128, 2, N], BF)
        src_sb = consts.tile([128, 2, N], BF)
        mask_sb = consts.tile([128, 16, N], BF)
        q_sb = consts.tile([128, 2, N], BF)
        k_sb = consts.tile([128, 2, N], BF)
        vt_sb = consts.tile([128, 16, H, DH + 1], BF)
        attn_sb = consts.tile([128, 2, N], BF)
        msg_sb = consts.tile([128, 2, N], BF)
        y1n_sb = consts.tile([128, 4, N], BF)
        y1a_sb = consts.tile([128, 2, 1024], BF)
        eps_sb = consts.tile([128, 1], F32)

        # ---- input DMA ----
        # each dma_start costs ~600ns of *serial issue time* on its queue's
        # sequencer, and the head is DMA-bandwidth-bound (13 MB of input),
        # so transfers are ordered strictly by first need: x/src halves on
        # the sync queue, mask quarters on the scalar HWDGE queue.  Mask
        # quarter 0 is split so chain tile mc only waits for its own rows.
        nc.sync.dma_start(out=wq_sb[:], in_=d_wq[:])
        nc.sync.dma_start(out=bq_sb[:], in_=d_bq[:])
        nc.sync.dma_start(out=x_sb[:, :, 0:1024], in_=d_x[:, :, 0:1024])
        nc.sync.dma_start(out=wk_sb[:], in_=d_wk[:])
        nc.sync.dma_start(out=bk_sb[:], in_=d_bk[:])
        for mg in range(4):
            nc.scalar.dma_start(
                out=mask_sb[:, mg * 4:(mg + 1) * 4, 0:512],
                in_=d_mask[:, mg * 4:(mg + 1) * 4, 0:512])
        nc.sync.dma_start(out=src_sb[:, :, 0:1024], in_=d_src[:, :, 0:1024])
        nc.sync.dma_start(out=wv_sb[:], in_=d_wv[:])
        nc.sync.dma_start(out=src_sb[:, :, 1024:2048],
                          in_=d_src[:, :, 1024:2048])
        nc.sync.dma_start(out=x_sb[:, :, 1024:2048], in_=d_x[:, :, 1024:2048])
        for mh in range(2):
            nc.scalar.dma_start(
                out=mask_sb[:, mh * 8:(mh + 1) * 8, 512:1024],
                in_=d_mask[:, mh * 8:(mh + 1) * 8, 512:1024])
        for w_sb, d_w in ((wm_sb, d_wm), (w1_sb, d_w1), (w2_sb, d_w2),
                          (bm_sb, d_bm)):
            nc.sync.dma_start(out=w_sb[:], in_=d_w[:])
        # mask quarters 2/3 are issued from inside the pass loop so their
        # multi-MB transfers never block the scalar queue's ring ahead of
        # the projection evictions

        nc.vector.memset(eps_sb[:], EPS)
        nc.vector.memset(vt_sb[:, :, :, DH:DH + 1], 1.0)

        with tc.tile_pool(name="psA", bufs=2, space="PSUM") as psA, \
             tc.tile_pool(name="psB", bufs=4, space="PSUM") as psB:
            # ---- projections ----
            # q/k chunk 0 first, then vT, then chunk 1: attention on head
            # pair 0 can start as soon as chunk 0 and vT are out.
            def proj_qk(w_sb, b_sb, rhs_sb, dst, oc):
                for q4 in range(4):
                    pp = psB.tile([128, 512], F32, tag="psB")
                    n0 = q4 * 512
                    for kc in range(2):
                        nc.tensor.matmul(
                            pp[:],
                            lhsT=w_sb[:, kc, oc * 128:(oc + 1) * 128],
                            rhs=rhs_sb[:, kc, n0:n0 + 512],
                            start=(kc == 0), stop=(kc == 1))
                    nc.scalar.activation(
                        dst[:, oc, n0:n0 + 512], pp[:],
                        AF.Identity, bias=b_sb[:, oc:oc + 1])

            pass  # projections are emitted arrival-ordered below

            # vT: produced directly transposed, [m, o] per 128-chunk of m,
            # column DH of each head = ones (softmax denominator row).
            # evicted on the vector engine -- ACT is the scarce resource
            # once the score evictions start
            def make_vt(mc):
                pv = psB.tile([128, 256], F32, tag="psB")
                for kc in range(2):
                    nc.tensor.matmul(
                        pv[:],
                        lhsT=src_sb[:, kc, mc * 128:(mc + 1) * 128],
                        rhs=wv_sb[:, kc, :],
                        start=(kc == 0), stop=(kc == 1))
                nc.vector.tensor_copy(
                    vt_sb[:, mc, :, 0:DH],
                    pv[:].rearrange("p (h d) -> p h d", h=H))

            def proj_chunk(w_sb, b_sb, rhs_sb, dst, oc, q4):
                pp = psB.tile([128, 512], F32, tag="psB")
                n0 = q4 * 512
                for kc in range(2):
                    nc.tensor.matmul(
                        pp[:],
                        lhsT=w_sb[:, kc, oc * 128:(oc + 1) * 128],
                        rhs=rhs_sb[:, kc, n0:n0 + 512],
                        start=(kc == 0), stop=(kc == 1))
                nc.scalar.activation(
                    dst[:, oc, n0:n0 + 512], pp[:],
                    AF.Identity, bias=b_sb[:, oc:oc + 1])

            def q_chunk(oc, q4):
                proj_chunk(wq_sb, bq_sb, x_sb, q_sb, oc, q4)

            def k_chunk(oc, q4):
                proj_chunk(wk_sb, bk_sb, src_sb, k_sb, oc, q4)

            # pre-pass projections ordered by DMA arrival so the PE FIFO
            # never waits on a transfer that a later-needed chunk depends on;
            # everything else is injected into pass 0
            q_chunk(0, 0)
            k_chunk(0, 0)
            q_chunk(0, 1)
            k_chunk(0, 1)
            for mc in range(8):
                make_vt(mc)
            k_chunk(0, 2)
            k_chunk(0, 3)
            for mc in range(8, 16):
                make_vt(mc)

            # ---- attention (scores transposed: [m, n]) ----
            # Head pairs packed into the full PE array via row tiling.
            # Software-pipelined: attention matmuls trail the scores matmuls
            # by up to 3 iterations; accumulator drains are deferred into
            # the next pass.
            passes = [(0, 0), (0, 1), (1, 0), (1, 1),
                      (0, 2), (0, 3), (1, 2), (1, 3)]
            pending = []            # (pi, off, ap_e, ap_o, hc, mc)
            epilogue = None         # (ap_e, ap_o, hc, nq4, pi)
            epi_mid = None          # ((stg, rtmp) pairs, hc, nq4, pi)

            def flush_attn():
                pi, off, ap_e, ap_o, hc, mc = pending.pop(0)
                nc.tensor.matmul(
                    ap_e[:], lhsT=vt_sb[:, mc, 2 * hc, :],
                    rhs=pi[:, off:off + 512].bitcast(BF),
                    start=(mc == 0), stop=(mc == 15))
                nc.tensor.matmul(
                    ap_o[:], lhsT=vt_sb[:, mc, 2 * hc + 1, :],
                    rhs=pi[:, off + 512:off + 1024].bitcast(BF),
                    start=(mc == 0), stop=(mc == 15))

            # the epilogue is drained in two steps: the accumulator staging
            # (which releases the psB slots) plus the sum-row DMA round trip
            # start early at mc==1; the reciprocal + normalize run at mc==5
            # so the DVE never sits in-FIFO waiting on the DRAM round trip
            def flush_epilogue():
                nonlocal epilogue, epi_mid
                if epilogue is None:
                    return
                ap_e, ap_o, hc, nq4, pi_ = epilogue
                epilogue = None
                mids = []
                for side, ap_t in ((0, ap_e), (1, ap_o)):
                    ri = pi_ * 2 + side
                    # stage the whole accumulator (attn rows + exp-sum row)
                    stg = stgp.tile([65, 512], F32, tag="stg")
                    nc.scalar.activation(stg[:], ap_t[:], AF.Copy)
                    # reshape [1,512]->[128,4] through DRAM so the divide
                    # runs on 128 lanes
                    nc.sync.dma_start(out=d_sums[ri:ri + 1, :],
                                      in_=stg[64:65, :])
                    rtmp = recp.tile([128, 4], F32, tag="rtmp")
                    nc.sync.dma_start(
                        out=rtmp[:],
                        in_=d_sums[ri:ri + 1, :].rearrange(
                            "a (p c) -> (a p) c", p=128))
                    mids.append((stg, rtmp))
                epi_mid = (mids, hc, nq4, pi_)

            def finish_epilogue():
                nonlocal epi_mid
                if epi_mid is None:
                    return
                mids, hc, nq4, pi_ = epi_mid
                epi_mid = None
                n0 = nq4 * 512
                for side, (stg, rtmp) in enumerate(mids):
                    hp = side * 64
                    ri = pi_ * 2 + side
                    rcp = recp.tile([128, 4], F32, tag="rcp")
                    nc.vector.reciprocal(rcp[:], rtmp[:])
                    nc.sync.dma_start(
                        out=d_rscr[ri:ri + 1, :].rearrange(
                            "a (p c) -> (a p) c", p=128),
                        in_=rcp[:])
                    rsc = d_rscr.ap()
                    bcast = bass.AP(tensor=rsc.tensor, offset=ri * 512,
                                    ap=[[0, 64], [1, 512]])
                    rb = rbb.tile([64, 512], F32, tag="rb")
                    nc.sync.dma_start(out=rb[:], in_=bcast)
                    nc.gpsimd.tensor_tensor(
                        attn_sb[hp:hp + 64, hc, n0:n0 + 512],
                        stg[0:64, :], rb[:], op=ALU.mult)

            # flat tile loop: scores are emitted LEAD tiles ahead of the
            # chain work so the next pass's scores sit in the PE FIFO ahead
            # of the previous pass's trailing attention matmuls (kills the
            # per-pass-boundary pipeline bubble)
            LEAD = 2
            sp_tiles = {}
            ap_cur = [None, None]
            pt_pair = None
            pi_pair = None

            def emit_scores(j):
                pj, mcj = j // 16, j % 16
                hcj, nqj = passes[pj]
                nj = nqj * 512
                sp = psA.tile([128, 1024], F32, tag="psA")
                sp_tiles[j] = sp
                nc.tensor.matmul(
                    sp[:, 0:512],
                    lhsT=k_sb[0:64, hcj, mcj * 128:(mcj + 1) * 128],
                    rhs=q_sb[0:64, hcj, nj:nj + 512],
                    tile_position=(0, 0))
                nc.tensor.matmul(
                    sp[:, 512:1024],
                    lhsT=k_sb[64:128, hcj, mcj * 128:(mcj + 1) * 128],
                    rhs=q_sb[64:128, hcj, nj:nj + 512],
                    tile_position=(64, 0))

            for j in range(LEAD):
                emit_scores(j)
            for i in range(128):
                pi_, mc = i // 16, i % 16
                hc, nq4 = passes[pi_]
                n0 = nq4 * 512
                sp = sp_tiles.pop(i)
                while len(pending) >= 3:
                    flush_attn()
                if mc == 1:
                    flush_epilogue()
                    # the new accumulator pair is allocated only after the
                    # previous pass's drain is emitted, so the pool reuse
                    # dependency is recorded correctly
                    ap_cur = [psB.tile([65, 512], F32, tag="psB", name="ape"),
                              psB.tile([65, 512], F32, tag="psB", name="apo")]
                if mc == 5:
                    finish_epilogue()
                if mc in (3, 7, 11, 15) and pi_ in (1, 2):
                    # trickle the late mask quarters onto the scalar queue
                    q4m = pi_ + 1
                    mg = mc // 4
                    nc.scalar.dma_start(
                        out=mask_sb[:, mg * 4:(mg + 1) * 4,
                                    q4m * 512:(q4m + 1) * 512],
                        in_=d_mask[:, mg * 4:(mg + 1) * 4,
                                   q4m * 512:(q4m + 1) * 512])
                ap_e, ap_o = ap_cur
                if True:
                    mrow = mask_sb[:, mc, n0:n0 + 512]
                    mb = bass.AP(tensor=mrow.tensor, offset=mrow.offset,
                                 ap=[list(mrow.ap[0]), [0, 2], [1, 512]])
                    if mc % 2 == 0:
                        pt_pair = ptp.tile([128, 2048], BF, tag="pt")
                        pi_pair = pip.tile([128, 2048], I16, tag="pi")
                    half = (mc % 2) * 1024
                    pt_h = pt_pair[:, half:half + 1024]
                    if mc in F_SET:
                        # chain F: DVE multiply straight from PSUM (1x)
                        nc.vector.tensor_tensor(
                            pt_h.rearrange("p (t n) -> p t n", t=2),
                            sp[:].rearrange("p (t n) -> p t n", t=2),
                            mb, op=ALU.mult)
                    else:
                        # chain E: ACT evicts PSUM->SBUF bf16; DVE multiply
                        # in 2x bf16 mode
                        spb = spbp.tile([128, 1024], BF, tag="spb")
                        nc.scalar.activation(spb[:], sp[:], AF.Copy)
                        nc.vector.tensor_tensor(
                            pt_h.rearrange("p (t n) -> p t n", t=2),
                            spb[:].rearrange("p (t n) -> p t n", t=2),
                            mb, op=ALU.mult)
                    if mc % 2 == 1:
                        # one magic add (4x mode) covers both tiles of the
                        # pair; int16 out, bitcast to bf16 by the consumer
                        nc.vector.tensor_scalar(pi_pair[:], pt_pair[:],
                                                MAGIC, None, op0=ALU.add)
                        pending.append((pi_pair, 0, ap_e, ap_o, hc, mc - 1))
                        pending.append((pi_pair, 1024, ap_e, ap_o, hc, mc))
                    if pi_ == 0 and mc in (2, 3):
                        q_chunk(0, mc)  # q head-pair 0, n >= 1024
                    if pi_ == 0 and mc in (4, 6, 8, 10):
                        j = (mc - 4) // 2
                        proj_chunk(wq_sb, bq_sb, x_sb, q_sb, 1, j)
                    if pi_ == 0 and mc in (5, 7, 9, 11):
                        j = (mc - 5) // 2
                        proj_chunk(wk_sb, bk_sb, src_sb, k_sb, 1, j)
                    if mc in (6, 10) and pi_ in (4, 5):
                        # merge conv for the first n-half, two 512-chunks
                        # per pass in psB slots freed by the early epilogue
                        # flush (attn h0 is complete by pass 4)
                        j = (pi_ - 4) * 2 + (mc == 10)
                        oc, nq = j // 2, j % 2
                        mp = psB.tile([128, 512], F32, tag="psB")
                        for kc in range(2):
                            nc.tensor.matmul(
                                mp[:],
                                lhsT=wm_sb[:, kc, oc * 128:(oc + 1) * 128],
                                rhs=attn_sb[:, kc, nq * 512:(nq + 1) * 512],
                                start=(kc == 0), stop=(kc == 1))
                        nc.scalar.activation(
                            msg_sb[:, oc, nq * 512:(nq + 1) * 512],
                            mp[:], AF.Identity, bias=bm_sb[:, oc:oc + 1])
                    if mc in (6, 10) and pi_ in (6, 7):
                        # first n-half of y1 for oc 0/1, evicted to SBUF
                        # bf16 (stats are taken in the tail)
                        j = (pi_ - 6) * 2 + (mc == 10)
                        oc, nq = j // 2, j % 2
                        yp = psB.tile([128, 512], F32, tag="psB")
                        for kc in range(4):
                            rhs_sb = x_sb if kc < 2 else msg_sb
                            nc.tensor.matmul(
                                yp[:],
                                lhsT=w1_sb[:, kc, oc * 128:(oc + 1) * 128],
                                rhs=rhs_sb[:, kc % 2,
                                           nq * 512:(nq + 1) * 512],
                                start=(kc == 0), stop=(kc == 3))
                        nc.scalar.activation(
                            y1a_sb[:, oc, nq * 512:(nq + 1) * 512],
                            yp[:], AF.Copy)
                if mc == 15:
                    epilogue = (ap_e, ap_o, hc, nq4, pi_)
                if i + LEAD < 128:
                    emit_scores(i + LEAD)
            while pending:
                flush_attn()
            flush_epilogue()
            finish_epilogue()

        with tc.tile_pool(name="psM", bufs=8, space="PSUM") as psM:
            # ---- merge conv + MLP1 + InstanceNorm/ReLU, all at 512-wide
            # quarter granularity so the n<1536 work runs during the last
            # epilogue's DMA round trip and only the final quarter chains
            # behind it ----
            y1_stats = {}

            def merge_q(oc, q):
                mp = psM.tile([128, 512], F32, tag="psM")
                for kc in range(2):
                    nc.tensor.matmul(
                        mp[:],
                        lhsT=wm_sb[:, kc, oc * 128:(oc + 1) * 128],
                        rhs=attn_sb[:, kc, q * 512:(q + 1) * 512],
                        start=(kc == 0), stop=(kc == 1))
                nc.scalar.activation(
                    msg_sb[:, oc, q * 512:(q + 1) * 512],
                    mp[:], AF.Identity, bias=bm_sb[:, oc:oc + 1])

            # one y1 quarter: matmuls, stats from PSUM, then staged into
            # y1n_sb (pre-norm values; the ReLU below rewrites in place)
            def y1_q(oc, q):
                if oc not in y1_stats:
                    st_new = statp.tile([128, 4, 6], F32, tag="st")
                    y1_stats[oc] = st_new
                st = y1_stats[oc]
                yp = psM.tile([128, 512], F32, tag="psM")
                n0 = q * 512
                for kc in range(4):
                    rhs_sb = x_sb if kc < 2 else msg_sb
                    nc.tensor.matmul(
                        yp[:],
                        lhsT=w1_sb[:, kc, oc * 128:(oc + 1) * 128],
                        rhs=rhs_sb[:, kc % 2, n0:n0 + 512],
                        start=(kc == 0), stop=(kc == 3))
                nc.vector.bn_stats(st[:, q, :], yp[:])
                nc.scalar.activation(y1n_sb[:, oc, n0:n0 + 512], yp[:],
                                     AF.Copy)

            def y1_norm(oc):
                st = y1_stats[oc]
                mv = statp.tile([128, 2], F32, tag="mv")
                nc.vector.bn_aggr(mv[:], st[:])
                sq = statp.tile([128, 1], F32, tag="sq")
                nc.scalar.activation(sq[:], mv[:, 1:2], AF.Sqrt,
                                     bias=eps_sb[:])
                rs = statp.tile([128, 1], F32, tag="rs")
                nc.vector.reciprocal(rs[:], sq[:])
                nb = statp.tile([128, 1], F32, tag="nb")
                nc.vector.scalar_tensor_tensor(nb[:], mv[:, 0:1], -1.0, rs[:],
                                               op0=ALU.mult, op1=ALU.mult)
                for half in range(2):
                    h0, h1 = half * 1024, (half + 1) * 1024
                    if oc < 2 and half == 0:
                        src_ap = y1a_sb[:, oc, :]  # injected h0, SBUF bf16
                    else:
                        src_ap = y1n_sb[:, oc, h0:h1]  # in-place
                    nc.scalar.activation(
                        y1n_sb[:, oc, h0:h1], src_ap, AF.Relu,
                        bias=nb[:], scale=rs[:])

            def mlp2_q(oc, q):
                op_t = psM.tile([128, 512], F32, tag="psM")
                n0 = q * 512
                for kc in range(4):
                    nc.tensor.matmul(
                        op_t[:],
                        lhsT=w2_sb[:, kc, oc * 128:(oc + 1) * 128],
                        rhs=y1n_sb[:, kc, n0:n0 + 512],
                        start=(kc == 0), stop=(kc == 3))
                o_sb = outp.tile([128, 512], F32, tag="outsb")
                # staged on the vector engine: ACT is busy with the ReLU
                # stream at this point, DVE is idle
                nc.vector.tensor_copy(o_sb[:], op_t[:])
                nc.sync.dma_start(out=d_out[:, oc, n0:n0 + 512], in_=o_sb[:])

            # stats for the injected oc 0/1 first half (SBUF bf16)
            for oc in range(2):
                st_new = statp.tile([128, 4, 6], F32, tag="st")
                y1_stats[oc] = st_new
                for nq in range(2):
                    nc.vector.bn_stats(st_new[:, nq, :],
                                       y1a_sb[:, oc, nq * 512:(nq + 1) * 512])
            # everything not gated on the final attention quarter first
            merge_q(0, 2)
            merge_q(1, 2)
            y1_q(2, 0)
            y1_q(2, 1)
            y1_q(3, 0)
            y1_q(3, 1)
            y1_q(0, 2)
            y1_q(1, 2)
            y1_q(2, 2)
            y1_q(3, 2)
            # the final quarter (n 1536:2048), gated on the last epilogue
            merge_q(0, 3)
            merge_q(1, 3)
            y1_q(0, 3)
            y1_norm(0)
            y1_q(1, 3)
            y1_norm(1)
            y1_q(2, 3)
            y1_norm(2)
            y1_q(3, 3)
            y1_norm(3)
            for q in range(4):
                mlp2_q(0, q)
                mlp2_q(1, q)

    nc.compile()
    return nc


def _chunk(a, p=128):
    # [C, ...] -> [128, C//128, ...] with partition-major layout
    c = a.shape[0]
    return np.ascontiguousarray(
        a.reshape(c // p, p, *a.shape[1:]).swapaxes(0, 1))


def _prep_inputs(x, source, mask, Wq, bq, Wk, bk, Wv, bv, Wm, bm, W1, b1,
                 W2, b2):
    # blocked-head channel permutation: new[h*64+d] = old[d*4+h]
    perm = (np.arange(DH)[None, :] * H + np.arange(H)[:, None]).reshape(-1)
    # fold 1/sqrt(dh) and the 2^7/ln2 exp2 scale into the q projection
    scale = 128.0 / (np.sqrt(np.float32(DH)) * np.log(2.0))

    wq_t = _chunk((Wq[perm, :] * scale).T.astype(NPBF))
    wk_t = _chunk(Wk[perm, :].T.astype(NPBF))
    wv_t = _chunk(Wv[perm, :].T.astype(NPBF))
    wm_t = _chunk(Wm[:, perm].T.astype(NPBF))
    w1_t = _chunk(W1.T.astype(NPBF))
    w2_t = _chunk(W2.T.astype(NPBF))
    bq_t = _chunk((bq[perm] * scale).astype(np.float32))
    bk_t = _chunk(bk[perm].astype(np.float32))
    bm_t = _chunk((Wm @ bv + bm).astype(np.float32))

    shared = {"wqT": wq_t, "wkT": wk_t, "wvT": wv_t, "wmT": wm_t,
              "w1T": w1_t, "w2T": w2_t, "bq": bq_t, "bk": bk_t, "bmE": bm_t}

    in_maps = []
    for b in range(B):
        m = dict(shared)
        m["x"] = _chunk(np.asarray(x[b]).astype(NPBF))
        m["src"] = _chunk(np.asarray(source[b]).astype(NPBF))
        m["maskT"] = _chunk(np.ascontiguousarray(
            np.asarray(mask[b]).T).astype(NPBF))
        in_maps.append(m)
    return in_maps


def run(inputs, trace=False, tmpdir=None):
    if "nc" not in _CACHE:
        _CACHE["nc"] = _build()
    nc = _CACHE["nc"]
    in_maps = _prep_inputs(**inputs)
    res = run_bass_kernel_spmd(nc, in_maps, list(range(NCORES)), trace=trace,
                               tmpdir=tmpdir)
    out = np.empty((B, D, N), np.float32)
    for b in range(B):
        o = res.results[b]["out"]  # [128, 2, N]
        out[b] = o.swapaxes(0, 1).reshape(D, N)
    return out, res


def kernel(**inputs):
    out, _ = run(inputs, trace=False)
    return out

